# revision 1
# baseline (speedup 1.0000x reference)
"""Trainium2 Bass kernel for nn_Encoder_82575041233042.

6-layer weight-shared pre-LN transformer encoder, B=2, S=2048, D=1024,
H=16 heads (d_k=64), FF=4096, fp32 I/O, mask all-ones.

Sharding: 8-way row-parallel over the 4096 (batch*seq) token rows; each
core owns 512 contiguous rows of one batch element (cores 0-3 <-> batch
0, cores 4-7 <-> batch 1). Per layer each core computes K/V for its own
rows, AllGathers K/V (bf16) within its 4-core group, then runs the whole
layer (attention over the full 2048-key sequence + FFN) for its own rows
only. The final output is assembled on the host.

Attention uses a transposed-scores layout: S^T[k, q] tiles come straight
from matmul (keys on partitions, queries on the free axis), exp runs on
ScalarE with the 1/sqrt(d_k) folded into the activation scale, and the
softmax denominator comes for free from a ones-column appended to V
(row 64 of the attnV accumulation = sum_k P). The division by the
denominator uses an Exp(-Ln(l)) reciprocal (same ACT table set as the
softmax exp) and a tiny K=2 PE matmul that broadcasts 1/l across
partitions. No max-subtraction: scores here are O(1) (LN'd activations
times 0.02-scale weights), far inside exp's fp32 range.

Matmuls run in bf16 with fp32 PSUM accumulation.
"""

import sys

if "/opt/trn_rl_repo" not in sys.path:
    sys.path.insert(0, "/opt/trn_rl_repo")

import numpy as np
import ml_dtypes

import bass_rust
import concourse.bass as bass
import concourse.mybir as mybir
import concourse.tile as tile
from concourse.bass_utils import run_bass_kernel_spmd

# ---------------------------------------------------------------------------
# Workaround: this walrus build rejects more than ONE sync wait per
# instruction ("Too many sync wait commands" in CoreV3GenImpl::setupSyncWait).
# Post-pass: any instruction carrying N>1 sem waits gets N-1 same-engine
# NoOps inserted immediately before it, each carrying one of the extra
# waits. Engine streams execute sequentially, so AND-semantics hold.
# ---------------------------------------------------------------------------

def _split_multiwaits(nc):
    all_created = set()
    for f in nc.m.functions:
        for blk in list(f.blocks):
            insts = [i for i in blk.instructions if i.name not in all_created]
            plans = {}
            for idx, inst in enumerate(insts):
                si = inst.sync_info
                if si is not None and si.on_wait and len(si.on_wait) > 1:
                    waits = list(si.on_wait)
                    nops = []
                    for w in waits[:-1]:
                        nop = nc.engines[inst.engine].nop().ins
                        nop.sync_info = bass_rust.SyncInfo(on_wait=[w], on_update=[])
                        nops.append(nop)
                        all_created.add(nop.name)
                    si.on_wait = waits[-1:]
                    plans[idx] = nops
            if plans:
                new = []
                for idx, inst in enumerate(insts):
                    if idx in plans:
                        new.extend(plans[idx])
                    new.append(inst)
                blk.instructions = new
            else:
                blk.instructions = insts
    for f in nc.m.functions:
        for blk in f.blocks:
            seen = set()
            out = []
            for inst in blk.instructions:
                if inst.name in seen:
                    continue
                seen.add(inst.name)
                out.append(inst)
            blk.instructions = out
    return nc


# ---------------------------------------------------------------------------
B, S, D = 2, 2048, 1024
H, DK, FF = 16, 64, 4096
NL = 6
LN_EPS = 1e-5
NCORES = 8
GROUP = 4                 # cores per batch element
S_OWN = S * B // NCORES   # 512 token rows per core
P = 128
QT = S_OWN // P           # 4 q-tiles of own rows
CH = D // P               # 8 contraction chunks of d_model
FFCH = FF // P            # 32 ff chunks
KTILES = S // P           # 16 key tiles of the full sequence
PAIRS = H // 2            # 8 head pairs
HD = D // 2               # 512
KV_FLAT = S_OWN * D       # flat elems of one K^T / V own block

F32 = mybir.dt.float32
BF16 = mybir.dt.bfloat16
AF = mybir.ActivationFunctionType
ALU = mybir.AluOpType
AX = mybir.AxisListType


def _view(ap, *shape):
    flat = ap
    if len(flat.shape) > 1:
        dims = " ".join(f"a{i}" for i in range(len(flat.shape)))
        flat = flat.rearrange(f"{dims} -> ({dims})")
    names = " ".join(f"b{i}" for i in range(len(shape)))
    kw = {f"b{i}": s for i, s in enumerate(shape)}
    return flat.rearrange(f"({names}) -> {names}", **kw)


def build_program(nl=NL, debug=False):
    """Build the SPMD Bass program (identical on all 8 cores)."""
    nc = bass.Bass()
    dbg = {}
    if debug:
        dbg["xnt"] = nc.dram_tensor("dbg_xnt", [P, CH, S_OWN], BF16, kind="ExternalOutput")
        dbg["kvf"] = nc.dram_tensor("dbg_kvf", [GROUP, 2, KV_FLAT], BF16, kind="ExternalOutput")
        dbg["qt"] = nc.dram_tensor("dbg_qt", [P, PAIRS, S_OWN], BF16, kind="ExternalOutput")
        dbg["o"] = nc.dram_tensor("dbg_o", [P, PAIRS, S_OWN], BF16, kind="ExternalOutput")
        dbg["hattn"] = nc.dram_tensor("dbg_hattn", [P, QT, D], F32, kind="ExternalOutput")
        dbg["ht"] = nc.dram_tensor("dbg_ht", [P, FFCH, S_OWN], BF16, kind="ExternalOutput")
        dbg["h1"] = nc.dram_tensor("dbg_h1", [P, QT, D], F32, kind="ExternalOutput")
        dbg["pss0"] = nc.dram_tensor("dbg_pss0", [P, S_OWN], F32, kind="ExternalOutput")
        dbg["pt0"] = nc.dram_tensor("dbg_pt0", [P, S_OWN], BF16, kind="ExternalOutput")
        dbg["pso0"] = nc.dram_tensor("dbg_pso0", [DK + 1, S_OWN], F32, kind="ExternalOutput")
        dbg["linv0"] = nc.dram_tensor("dbg_linv0", [DK + 1, S_OWN], F32, kind="ExternalOutput")

    x_own = nc.dram_tensor("x_own", [S_OWN, D], F32, kind="ExternalInput")
    wq = nc.dram_tensor("wq", [D, D], BF16, kind="ExternalInput")
    wk = nc.dram_tensor("wk", [D, D], BF16, kind="ExternalInput")
    wv = nc.dram_tensor("wv", [D, D], BF16, kind="ExternalInput")
    wo = nc.dram_tensor("wo", [D, D], BF16, kind="ExternalInput")
    w1h = nc.dram_tensor("w1h", [FFCH, D, P], BF16, kind="ExternalInput")
    w2 = nc.dram_tensor("w2", [FF, D], BF16, kind="ExternalInput")
    e2 = nc.dram_tensor("e2", [DK + 1, P], F32, kind="ExternalInput")
    ident = nc.dram_tensor("ident", [P, P], BF16, kind="ExternalInput")
    out = nc.dram_tensor("out", [S_OWN, D], F32, kind="ExternalOutput")

    KVH = KV_FLAT // 2
    k_own = [[nc.dram_tensor(f"k_own_{i}_{hh}", [KVH], BF16) for hh in range(2)]
             for i in range(NL)]
    v_own = [[nc.dram_tensor(f"v_own_{i}_{hh}", [KVH], BF16) for hh in range(2)]
             for i in range(NL)]
    k_full = [[nc.dram_tensor(f"k_full_{i}_{hh}", [GROUP, KVH], BF16)
               for hh in range(2)] for i in range(NL)]
    v_full = [[nc.dram_tensor(f"v_full_{i}_{hh}", [GROUP, KVH], BF16)
               for hh in range(2)] for i in range(NL)]
    RG = [[0, 1, 2, 3], [4, 5, 6, 7]]

    with tile.TileContext(nc) as tc:
        with (
            tc.tile_pool(name="const", bufs=1) as cpool,
            tc.tile_pool(name="resw", bufs=1) as wpool,      # wo resident
            tc.tile_pool(name="hpool", bufs=1) as hpool,     # residual h
            tc.tile_pool(name="big", bufs=1) as bpool,       # xnt/qt/o/ht
            tc.tile_pool(name="small", bufs=2) as apool,     # LN/l scratch
            tc.tile_pool(name="wqkv", bufs=3) as qkvpool,    # per-pair w tiles
            tc.tile_pool(name="wffn", bufs=2) as ffnpool,    # w1/w2 stream
            tc.tile_pool(name="kvs", bufs=2) as kvpool,      # K/V sb tiles
            tc.tile_pool(name="vsb", bufs=4) as vpool,       # V tiles, 2 heads deep
            tc.tile_pool(name="pts", bufs=3) as ptpool,      # P^T tiles
            tc.tile_pool(name="psMM", bufs=2, space="PSUM") as psMM,
            tc.tile_pool(name="psS", bufs=2, space="PSUM") as psS,
            tc.tile_pool(name="psO", bufs=2, space="PSUM") as psO,
        ):
            ident_sb = cpool.tile([P, P], BF16, tag="ident")
            nc.sync.dma_start(ident_sb[:], ident[:])
            warm = psMM.tile([P, P], F32, tag="mm")
            for _ in range(60):
                nc.tensor.matmul(warm[:], ident_sb[:], ident_sb[:],
                                 start=True, stop=True)
            e2_sb = cpool.tile([DK + 1, P], F32, tag="e2")
            nc.sync.dma_start(e2_sb[:], e2[:])
            eps_sb = cpool.tile([P, 1], F32, tag="eps")
            nc.vector.memset(eps_sb[:], LN_EPS)

            wo_sb = wpool.tile([P, CH, D], BF16, tag="wo")
            nc.sync.dma_start(wo_sb[:], wo.rearrange("(c p) n -> p c n", p=P))
            wv_sb = wpool.tile([P, CH, D], BF16, tag="wv")
            nc.sync.dma_start(wv_sb[:], wv.rearrange("(c p) n -> p c n", p=P))

            wq_v = wq.rearrange("(c p) n -> p c n", p=P)
            wk_v = wk.rearrange("(c p) n -> p c n", p=P)

            h_sb = hpool.tile([P, QT, D], F32, tag="h")
            nc.sync.dma_start(h_sb[:], x_own.rearrange("(t p) d -> p t d", p=P))

            def layernorm_stats(hsl, tagp):
                """Return (negmu, rstd) tiles [P,1] f32 for one qtile row-slice."""
                s1 = apool.tile([P, 1], F32, tag=f"{tagp}_s1")
                nc.vector.reduce_sum(s1[:], hsl, axis=AX.X)
                sqd = apool.tile([P, D], BF16, tag="sq_scratch")
                s2 = apool.tile([P, 1], F32, tag=f"{tagp}_s2")
                nc.scalar.activation(sqd[:], hsl, AF.Square, accum_out=s2[:])
                negmu = apool.tile([P, 1], F32, tag=f"{tagp}_negmu")
                nc.vector.tensor_scalar_mul(negmu[:], s1[:], -1.0 / D)
                mu2 = apool.tile([P, 1], F32, tag=f"{tagp}_mu2")
                nc.vector.tensor_mul(mu2[:], negmu[:], negmu[:])
                var = apool.tile([P, 1], F32, tag=f"{tagp}_var")
                nc.vector.tensor_scalar(var[:], s2[:], 1.0 / D, None, ALU.mult)
                nc.vector.tensor_sub(var[:], var[:], mu2[:])
                lnv = apool.tile([P, 1], F32, tag=f"{tagp}_lnv")
                nc.scalar.activation(lnv[:], var[:], AF.Ln, bias=eps_sb[:])
                rstd = apool.tile([P, 1], F32, tag=f"{tagp}_rstd")
                nc.scalar.activation(rstd[:], lnv[:], AF.Exp, scale=-0.5)
                return negmu, rstd

            def layernorm_transpose(xnt):
                """LN(h) -> xnT [P(dm), CH, S_OWN] bf16 (scale=1 bias=0)."""
                for qt in range(QT):
                    hsl = h_sb[:, qt, :]
                    negmu, rstd = layernorm_stats(hsl, "ln")
                    for c in range(CH):
                        xb = apool.tile([P, P], BF16, tag="xn_blk")
                        nc.vector.tensor_scalar(
                            xb[:], hsl[:, c * P:(c + 1) * P],
                            negmu[:], rstd[:], ALU.add, ALU.mult,
                        )
                        pst = psMM.tile([P, P], BF16, tag="mm")
                        nc.tensor.transpose(pst[:], xb[:], ident_sb[:])
                        nc.vector.tensor_copy(xnt[:, c, qt * P:(qt + 1) * P], pst[:])

            for L in range(nl):
                with nc.named_scope(f"L{L}_ln1"):
                    xnt1 = bpool.tile([P, CH, S_OWN], BF16, tag="xnt")
                    layernorm_transpose(xnt1)
                    if debug and L == 0:
                        nc.sync.dma_start(dbg["xnt"][:], xnt1[:])

                # ---- K^T (pairs) and V (own rows) -> kv_own -> AllGather --
                with nc.named_scope(f"L{L}_kv"):
                    for pr in range(PAIRS):
                        hh, prh = divmod(pr, PAIRS // 2)
                        ktv = _view(k_own[L][hh], PAIRS // 2, P, S_OWN)
                        wkc = qkvpool.tile([P, CH, P], BF16, tag="wkc")
                        nc.sync.dma_start(wkc[:], wk_v[:, :, pr * P:(pr + 1) * P])
                        psk = psMM.tile([P, S_OWN], F32, tag="mm")
                        for c in range(CH):
                            nc.tensor.matmul(
                                psk[:], wkc[:, c, :], xnt1[:, c, :],
                                start=(c == 0), stop=(c == CH - 1),
                            )
                        ktev = kvpool.tile([P, S_OWN], BF16, tag="ktev")
                        nc.vector.tensor_copy(ktev[:], psk[:])
                        nc.sync.dma_start(ktv[prh], ktev[:])
                        if prh == PAIRS // 2 - 1:
                            nc.gpsimd.collective_compute(
                                "AllGather", ALU.bypass, replica_groups=RG,
                                ins=[k_own[L][hh][:]], outs=[k_full[L][hh][:]],
                            )
                    for t in range(QT):
                        hh, th = divmod(t, 2)
                        vv = _view(v_own[L][hh], 2, P, 2, HD)
                        for hf in range(2):
                            psv = psMM.tile([P, HD], F32, tag="mm")
                            for c in range(CH):
                                nc.tensor.matmul(
                                    psv[:], xnt1[:, c, t * P:(t + 1) * P],
                                    wv_sb[:, c, hf * HD:(hf + 1) * HD],
                                    start=(c == 0), stop=(c == CH - 1),
                                )
                            vev = kvpool.tile([P, HD], BF16, tag="vev")
                            nc.vector.tensor_copy(vev[:], psv[:])
                            nc.sync.dma_start(vv[th, :, hf, :], vev[:])
                        if th == 1:
                            nc.gpsimd.collective_compute(
                                "AllGather", ALU.bypass, replica_groups=RG,
                                ins=[v_own[L][hh][:]], outs=[v_full[L][hh][:]],
                            )
                    if debug and L == 0:
                        for hh in range(2):
                            nc.sync.dma_start(
                                _view(dbg["kvf"], GROUP, 2, 2, KVH)[:, 0, hh, :],
                                k_full[L][hh][:])
                            nc.sync.dma_start(
                                _view(dbg["kvf"], GROUP, 2, 2, KVH)[:, 1, hh, :],
                                v_full[L][hh][:])

                # ---- Q^T (pairs), overlaps the gather ---------------------
                with nc.named_scope(f"L{L}_q"):
                    qt_sb = bpool.tile([P, PAIRS, S_OWN], BF16, tag="qt_sb")
                    for pr in range(PAIRS):
                        wqc = qkvpool.tile([P, CH, P], BF16, tag="wqc")
                        nc.sync.dma_start(wqc[:], wq_v[:, :, pr * P:(pr + 1) * P])
                        psq = psMM.tile([P, S_OWN], F32, tag="mm")
                        for c in range(CH):
                            nc.tensor.matmul(
                                psq[:], wqc[:, c, :], xnt1[:, c, :],
                                start=(c == 0), stop=(c == CH - 1),
                            )
                        nc.vector.tensor_copy(qt_sb[:, pr, :], psq[:])
                    if debug and L == 0:
                        nc.sync.dma_start(dbg["qt"][:], qt_sb[:])

                # ---- attention -------------------------------------------
                with nc.named_scope(f"L{L}_attn"):
                    o_sb = bpool.tile([P, PAIRS, S_OWN], BF16, tag="o_sb")
                    for pr in range(PAIRS):
                        kt_sb = kvpool.tile([P, GROUP, S_OWN], BF16, tag="kt_sb")
                        for b in range(GROUP):
                            nc.sync.dma_start(
                                kt_sb[:, b, :],
                                _view(k_full[L][pr // 4][b],
                                      PAIRS // 2, P, S_OWN)[pr % 4],
                            )
                        l2 = apool.tile([DK + 1, S_OWN], F32, tag="l2")
                        nc.vector.memset(l2[:], 1.0)
                        pso_pair = []
                        for par in range(2):
                            hd = pr * 2 + par
                            v_ab = []
                            for rh in range(2):
                                vt = vpool.tile([P, KTILES // 2, DK + 1], BF16,
                                                tag="v_sb")
                                nc.vector.memset(vt[:, :, DK:DK + 1], 1.0)
                                nc.sync.dma_start(
                                    vt[:, :, 0:DK],
                                    _view(v_full[L][rh], GROUP, 2, P, D)
                                    .rearrange("b t p d -> p (b t) d")[
                                        :, :, hd * DK:(hd + 1) * DK],
                                )
                                v_ab.append(vt)
                            pso = psO.tile([DK + 1, S_OWN], F32, tag="oo")
                            pso_pair.append(pso)
                            lo = par * DK
                            JORDER = [0, 4, 8, 12, 2, 6, 10, 14]
                            for i2, jbase in enumerate(JORDER):
                                pss = psS.tile([P, 2 * S_OWN], F32, tag="ss")
                                pt = ptpool.tile([P, 2 * S_OWN], BF16, tag="pt")
                                for u in range(2):
                                    j = jbase + u
                                    b, jj = divmod(j, QT)
                                    nc.tensor.matmul(
                                        pss[:, u * S_OWN:(u + 1) * S_OWN],
                                        kt_sb[lo:lo + DK, b, jj * P:(jj + 1) * P],
                                        qt_sb[lo:lo + DK, pr, :],
                                        start=True, stop=True,
                                    )
                                nc.scalar.activation(pt[:], pss[:], AF.Exp,
                                                     scale=0.125)
                                if debug and L == 0 and pr == 0 and par == 0 and j2 == 0:
                                    dcp = apool.tile([P, S_OWN], F32, tag="linv_sb")
                                    nc.vector.tensor_copy(dcp[:], pss[:, 0:S_OWN])
                                    nc.sync.dma_start(dbg["pss0"][:], dcp[:])
                                    nc.sync.dma_start(dbg["pt0"][:], pt[:, 0:S_OWN])
                                for u in range(2):
                                    j = jbase + u
                                    b, jj = divmod(j, QT)
                                    rh, jr = divmod(jj, 2)
                                    nc.tensor.matmul(
                                        pso[:], v_ab[rh][:, b * 2 + jr, :],
                                        pt[:, u * S_OWN:(u + 1) * S_OWN],
                                        start=(i2 == 0 and u == 0),
                                        stop=(i2 == len(JORDER) - 1 and u == 1),
                                    )
                            nc.vector.tensor_copy(l2[par * DK:par * DK + 1, :],
                                                  pso[DK:DK + 1, :])
                            if debug and L == 0 and pr == 0 and par == 0:
                                dcp2 = apool.tile([DK + 1, S_OWN], F32, tag="l2")
                                nc.vector.tensor_copy(dcp2[:], pso[:])
                                nc.sync.dma_start(dbg["pso0"][:], dcp2[:])
                        lnl = apool.tile([DK + 1, S_OWN], F32, tag="lnl")
                        nc.scalar.activation(lnl[:], l2[:], AF.Ln)
                        linv = apool.tile([DK + 1, S_OWN], F32, tag="linv")
                        nc.scalar.activation(linv[:], lnl[:], AF.Exp, scale=-1.0)
                        psl = psMM.tile([P, S_OWN], F32, tag="mm")
                        nc.tensor.matmul(psl[:], e2_sb[:], linv[:],
                                         start=True, stop=True)
                        if debug and L == 0 and pr == 0:
                            nc.sync.dma_start(dbg["linv0"][:], linv[:])
                        linv_sb = apool.tile([P, S_OWN], F32, tag="linv_sb")
                        nc.vector.tensor_copy(linv_sb[:], psl[:])
                        nc.vector.tensor_mul(
                            o_sb[0:DK, pr, :], pso_pair[0][0:DK, :],
                            linv_sb[0:DK, :],
                        )
                        nc.vector.tensor_mul(
                            o_sb[DK:P, pr, :], pso_pair[1][0:DK, :],
                            linv_sb[DK:P, :],
                        )

                if debug and L == 0:
                    nc.sync.dma_start(dbg["o"][:], o_sb[:])
                # ---- output projection + residual -------------------------
                with nc.named_scope(f"L{L}_oproj"):
                    for qt in range(QT):
                        for hf in range(2):
                            psa = psMM.tile([P, HD], F32, tag="mm")
                            for pr in range(PAIRS):
                                nc.tensor.matmul(
                                    psa[:], o_sb[:, pr, qt * P:(qt + 1) * P],
                                    wo_sb[:, pr, hf * HD:(hf + 1) * HD],
                                    start=(pr == 0), stop=(pr == PAIRS - 1),
                                )
                            hsl = h_sb[:, qt, hf * HD:(hf + 1) * HD]
                            nc.vector.tensor_add(hsl, hsl, psa[:])

                if debug and L == 0:
                    nc.sync.dma_start(dbg["hattn"][:], h_sb[:])
                # ---- FFN sublayer ----------------------------------------
                with nc.named_scope(f"L{L}_ln2"):
                    xnt2 = bpool.tile([P, CH, S_OWN], BF16, tag="xnt")
                    layernorm_transpose(xnt2)

                with nc.named_scope(f"L{L}_ffn1"):
                    ht_sb = bpool.tile([P, FFCH, S_OWN], BF16, tag="ht_sb")
                    for f in range(FFCH):
                        w1c = ffnpool.tile([P, CH, P], BF16, tag="w1c")
                        nc.sync.dma_start(
                            w1c[:], w1h[f].rearrange("(c p) f -> p c f", p=P)
                        )
                        psh = psMM.tile([P, S_OWN], F32, tag="mm")
                        for c in range(CH):
                            nc.tensor.matmul(
                                psh[:], w1c[:, c, :], xnt2[:, c, :],
                                start=(c == 0), stop=(c == CH - 1),
                            )
                        nc.vector.tensor_scalar_max(ht_sb[:, f, :], psh[:], 0.0)
                    if debug and L == 0:
                        nc.sync.dma_start(dbg["ht"][:], ht_sb[:])

                with nc.named_scope(f"L{L}_ffn2"):
                    for fo in range(4):
                        w2c = ffnpool.tile([P, CH, D], BF16, tag="w2c")
                        nc.sync.dma_start(
                            w2c[:],
                            w2[fo * CH * P:(fo + 1) * CH * P, :]
                            .rearrange("(c p) n -> p c n", p=P),
                        )
                        for qt in range(QT):
                            for hf in range(2):
                                psf = psMM.tile([P, HD], F32, tag="mm")
                                for fi in range(CH):
                                    f = fo * CH + fi
                                    nc.tensor.matmul(
                                        psf[:], ht_sb[:, f, qt * P:(qt + 1) * P],
                                        w2c[:, fi, hf * HD:(hf + 1) * HD],
                                        start=(fi == 0), stop=(fi == CH - 1),
                                    )
                                hsl = h_sb[:, qt, hf * HD:(hf + 1) * HD]
                                nc.vector.tensor_add(hsl, hsl, psf[:])

                if debug and L == 0:
                    nc.sync.dma_start(dbg["h1"][:], h_sb[:])

            # ---- final LN -> output ----------------------------------
            with nc.named_scope("lnf"):
                out_v = out.rearrange("(t p) d -> p t d", p=P)
                for qt in range(QT):
                    hsl = h_sb[:, qt, :]
                    negmu, rstd = layernorm_stats(hsl, "lnf")
                    ot = apool.tile([P, D], F32, tag="lnf_out")
                    nc.vector.tensor_scalar(
                        ot[:], hsl, negmu[:], rstd[:], ALU.add, ALU.mult
                    )
                    nc.sync.dma_start(out_v[:, qt, :], ot[:])

    _split_multiwaits(nc)
    return nc


_CACHED = {}


def _get_program():
    if "nc" not in _CACHED:
        _CACHED["nc"] = build_program()
    return _CACHED["nc"]


def make_in_maps(inputs):
    x = np.asarray(inputs["x"], np.float32)
    bf = ml_dtypes.bfloat16
    w1 = np.asarray(inputs["w1"], np.float32)
    w1hm = np.ascontiguousarray(
        w1.astype(bf).reshape(D, FFCH, P).transpose(1, 0, 2)
    )
    e2m = np.zeros((DK + 1, P), np.float32)
    e2m[0, 0:DK] = 1.0
    e2m[DK, DK:P] = 1.0
    common = {
        "wq": np.asarray(inputs["wq"], np.float32).astype(bf),
        "wk": np.asarray(inputs["wk"], np.float32).astype(bf),
        "wv": np.asarray(inputs["wv"], np.float32).astype(bf),
        "wo": np.asarray(inputs["wo"], np.float32).astype(bf),
        "w1h": w1hm,
        "w2": np.asarray(inputs["w2"], np.float32).astype(bf),
        "e2": e2m,
        "ident": np.eye(P, dtype=bf),
    }
    xr = x.reshape(B * S, D)
    in_maps = []
    for c in range(NCORES):
        m = dict(common)
        m["x_own"] = np.ascontiguousarray(xr[c * S_OWN:(c + 1) * S_OWN])
        in_maps.append(m)
    return in_maps


def kernel(**inputs):
    in_maps = make_in_maps(inputs)
    nc = _get_program()
    res = run_bass_kernel_spmd(nc, in_maps, list(range(NCORES)))
    full = np.concatenate([res.results[c]["out"] for c in range(NCORES)], axis=0)
    return full.reshape(B, S, D).astype(np.float32)



# revision 13
# speedup vs baseline: 1.0395x; 1.0395x over previous
"""Trainium2 Bass kernel for nn_Encoder_82575041233042.

6-layer weight-shared pre-LN transformer encoder, B=2, S=2048, D=1024,
H=16 heads (d_k=64), FF=4096, fp32 I/O, mask all-ones.

Sharding: 8-way row-parallel over the 4096 (batch*seq) token rows; each
core owns 512 contiguous rows of one batch element (cores 0-3 <-> batch
0, cores 4-7 <-> batch 1). Per layer each core computes K/V for its own
rows in fp8-e4m3, AllGathers K/V within its 4-core group (ring order
K-half0, V-half0, V-half1, K-half1 so attention can start early), then
runs the layer for its own rows. Output assembled on the host.

v2 changes vs baseline:
  - K/V/Q/P (softmax probs) in fp8-e4m3: halves collective bytes and
    K/V DMA; scores and attn*V matmuls run fp8xfp8.
  - Score matmuls for the two heads of a pair issued adjacently on
    disjoint 64-row groups -> concurrent on the PE array.
  - Softmax division deferred out of the attention loop: raw attn
    outputs and denominators are saved, one Ln/Exp + 8 broadcast
    matmuls normalize after the last pair.
  - LayerNorm stats via DVE bn_stats/bn_aggr; single fused 1024-wide
    (x-mu)*rstd apply per row-tile.
  - w2 + wv + wo resident in SBUF; FFN2 loops qt-outer so h row-tiles
    complete early and next-layer LN1 overlaps; stationary operands
    reused across hf halves in V/oproj/FFN2.

Matmuls accumulate in fp32 PSUM.
"""

import sys

if "/opt/trn_rl_repo" not in sys.path:
    sys.path.insert(0, "/opt/trn_rl_repo")

import numpy as np
import ml_dtypes

import bass_rust
import concourse.bass as bass
import concourse.mybir as mybir
import concourse.tile as tile
from concourse.bass_utils import run_bass_kernel_spmd

# ---------------------------------------------------------------------------
# Workaround: this walrus build rejects more than ONE sync wait per
# instruction. Post-pass: split multi-waits onto same-engine NoOps.
# ---------------------------------------------------------------------------

def _split_multiwaits(nc):
    all_created = set()
    for f in nc.m.functions:
        for blk in list(f.blocks):
            insts = [i for i in blk.instructions if i.name not in all_created]
            plans = {}
            for idx, inst in enumerate(insts):
                si = inst.sync_info
                if si is not None and si.on_wait and len(si.on_wait) > 1:
                    waits = list(si.on_wait)
                    nops = []
                    for w in waits[:-1]:
                        nop = nc.engines[inst.engine].nop().ins
                        nop.sync_info = bass_rust.SyncInfo(on_wait=[w], on_update=[])
                        nops.append(nop)
                        all_created.add(nop.name)
                    si.on_wait = waits[-1:]
                    plans[idx] = nops
            if plans:
                new = []
                for idx, inst in enumerate(insts):
                    if idx in plans:
                        new.extend(plans[idx])
                    new.append(inst)
                blk.instructions = new
            else:
                blk.instructions = insts
    for f in nc.m.functions:
        for blk in f.blocks:
            seen = set()
            out = []
            for inst in blk.instructions:
                if inst.name in seen:
                    continue
                seen.add(inst.name)
                out.append(inst)
            blk.instructions = out
    return nc


# ---------------------------------------------------------------------------
B, S, D = 2, 2048, 1024
H, DK, FF = 16, 64, 4096
NL = 6
LN_EPS = 1e-5
NCORES = 8
GROUP = 4                 # cores per batch element
S_OWN = S * B // NCORES   # 512 token rows per core
P = 128
QT = S_OWN // P           # 4 q-tiles of own rows
CH = D // P               # 8 contraction chunks of d_model
FFCH = FF // P            # 32 ff chunks
KTILES = S // P           # 16 key tiles of the full sequence
PAIRS = H // 2            # 8 head pairs
HD = D // 2               # 512
KV_FLAT = S_OWN * D       # flat elems of one K^T / V own block
KVH = KV_FLAT // 2        # one half (4 pairs of K, or one col-half of V)

F32 = mybir.dt.float32
BF16 = mybir.dt.bfloat16
FP8 = mybir.dt.float8e4
AF = mybir.ActivationFunctionType
ALU = mybir.AluOpType
AX = mybir.AxisListType


def _view(ap, *shape):
    flat = ap
    if len(flat.shape) > 1:
        dims = " ".join(f"a{i}" for i in range(len(flat.shape)))
        flat = flat.rearrange(f"{dims} -> ({dims})")
    names = " ".join(f"b{i}" for i in range(len(shape)))
    kw = {f"b{i}": s for i, s in enumerate(shape)}
    return flat.rearrange(f"({names}) -> {names}", **kw)


def build_program(nl=NL):
    """Build the SPMD Bass program (identical on all 8 cores)."""
    nc = bass.Bass()

    x_own = nc.dram_tensor("x_own", [S_OWN, D], F32, kind="ExternalInput")
    wq = nc.dram_tensor("wq", [D, D], BF16, kind="ExternalInput")
    wk = nc.dram_tensor("wk", [D, D], BF16, kind="ExternalInput")
    wv = nc.dram_tensor("wv", [D, D], BF16, kind="ExternalInput")
    wo = nc.dram_tensor("wo", [D, D], BF16, kind="ExternalInput")
    w1h = nc.dram_tensor("w1h", [FFCH, D, P], BF16, kind="ExternalInput")
    w2 = nc.dram_tensor("w2", [FF, D], BF16, kind="ExternalInput")
    e2 = nc.dram_tensor("e2", [DK + 1, P], F32, kind="ExternalInput")
    ident = nc.dram_tensor("ident", [P, P], BF16, kind="ExternalInput")
    out = nc.dram_tensor("out", [S_OWN, D], F32, kind="ExternalOutput")

    # internal K/V rings (fp8)
    k_own = [[nc.dram_tensor(f"k_own_{i}_{hh}", [KVH], FP8) for hh in range(2)]
             for i in range(nl)]
    v_own = [[nc.dram_tensor(f"v_own_{i}_{hf}", [KVH], FP8) for hf in range(2)]
             for i in range(nl)]
    k_full = [[nc.dram_tensor(f"k_full_{i}_{hh}", [GROUP, KVH], FP8)
               for hh in range(2)] for i in range(nl)]
    v_full = [[nc.dram_tensor(f"v_full_{i}_{hf}", [GROUP, KVH], FP8)
               for hf in range(2)] for i in range(nl)]
    RG = [[0, 1, 2, 3], [4, 5, 6, 7]]

    wq_v = wq.rearrange("(c p) n -> p c n", p=P)
    wk_v = wk.rearrange("(c p) n -> p c n", p=P)

    with tile.TileContext(nc) as tc:
        with (
            tc.tile_pool(name="const", bufs=1) as cpool,
            tc.tile_pool(name="resw", bufs=1) as wpool,      # wv/wo/w2 resident
            tc.tile_pool(name="hpool", bufs=1) as hpool,     # residual h
            tc.tile_pool(name="big", bufs=1) as bpool,       # xnt/qt/o/ht
            tc.tile_pool(name="small", bufs=2) as apool,     # LN scratch
            tc.tile_pool(name="lbuf", bufs=1) as lpool,      # per-layer scratch
            tc.tile_pool(name="wqkv", bufs=2) as qkvpool,    # wq/wk pair tiles
            tc.tile_pool(name="wffn", bufs=2) as ffnpool,    # w1 stream
            tc.tile_pool(name="kvs", bufs=2) as kvpool,      # K/V evac + kt
            tc.tile_pool(name="vsb", bufs=3) as vpool,       # V tiles
            tc.tile_pool(name="pts", bufs=2) as ptpool,      # P^T tiles
            tc.tile_pool(name="psMM", bufs=2, space="PSUM") as psMM,
            tc.tile_pool(name="psS", bufs=2, space="PSUM") as psS,
            tc.tile_pool(name="psO", bufs=2, space="PSUM") as psO,
        ):
            ident_sb = cpool.tile([P, P], BF16, tag="ident")
            nc.sync.dma_start(ident_sb[:], ident[:])
            warm = psMM.tile([P, P], F32, tag="mm")
            for _ in range(60):
                nc.tensor.matmul(warm[:], ident_sb[:], ident_sb[:],
                                 start=True, stop=True)
            e2_sb = cpool.tile([DK + 1, P], F32, tag="e2")
            nc.sync.dma_start(e2_sb[:], e2[:])

            eps_sb = cpool.tile([P, 1], F32, tag="eps")
            nc.vector.memset(eps_sb[:], LN_EPS)

            w2r = wpool.tile([P, FFCH, D], BF16, tag="w2r")
            nc.sync.dma_start(w2r[:], w2.rearrange("(f p) n -> p f n", p=P))

            h_sb = hpool.tile([P, QT, D], F32, tag="h")
            nc.sync.dma_start(h_sb[:], x_own.rearrange("(t p) d -> p t d", p=P))

            def layernorm_tile(hsl, tagp):
                """(negmu, rstd) [P,1] f32 for one [P, D] row-tile via bn_stats."""
                bns = apool.tile([P, 2, 6], F32, tag=f"{tagp}_bns")
                nc.vector.bn_stats(bns[:, 0, :], hsl[:, 0:HD])
                nc.vector.bn_stats(bns[:, 1, :], hsl[:, HD:D])
                mv = apool.tile([P, 2], F32, tag=f"{tagp}_mv")
                nc.vector.bn_aggr(mv[:], bns[:])
                negmu = apool.tile([P, 1], F32, tag=f"{tagp}_negmu")
                nc.vector.tensor_scalar_mul(negmu[:], mv[:, 0:1], -1.0)
                lnv = apool.tile([P, 1], F32, tag=f"{tagp}_lnv")
                nc.scalar.activation(lnv[:], mv[:, 1:2], AF.Ln, bias=eps_sb[:])
                rstd = apool.tile([P, 1], F32, tag=f"{tagp}_rstd")
                nc.scalar.activation(rstd[:], lnv[:], AF.Exp, scale=-0.5)
                return negmu, rstd

            def layernorm_transpose(xnt):
                """LN(h) -> xnT [P(dm), CH, S_OWN] bf16 (scale=1 bias=0)."""
                for qt in range(QT):
                    hsl = h_sb[:, qt, :]
                    negmu, rstd = layernorm_tile(hsl, "ln")
                    xb = apool.tile([P, D], BF16, tag="xn_blk",
                                    bufs=1)
                    nc.vector.tensor_scalar(
                        xb[:], hsl, negmu[:], rstd[:], ALU.add, ALU.mult,
                    )
                    for c in range(CH):
                        pst = psMM.tile([P, P], BF16, tag="mm")
                        nc.tensor.transpose(pst[:], xb[:, c * P:(c + 1) * P],
                                            ident_sb[:])
                        nc.vector.tensor_copy(xnt[:, c, qt * P:(qt + 1) * P],
                                              pst[:])

            for L in range(nl):
                # ---- LN1 -> xnt1 -------------------------------------------
                with nc.named_scope(f"L{L}_ln1"):
                    wv_sb = wpool.tile([P, CH, D], BF16, tag="wvo",
                                       name=f"wv_sb{L}")
                    nc.sync.dma_start(wv_sb[:],
                                      wv.rearrange("(c p) n -> p c n", p=P))
                    xnt1 = bpool.tile([P, CH, S_OWN], BF16, tag="xnt")
                    layernorm_transpose(xnt1)

                # ---- K^T pairs 0-3 -> gather K half0; V; K 4-7; gather ----
                def k_pair(pr):
                    hh, prh = divmod(pr, PAIRS // 2)
                    ktv = _view(k_own[L][hh], PAIRS // 2, P, S_OWN)
                    wkc = qkvpool.tile([P, CH, P], BF16, tag="wqkc")
                    nc.sync.dma_start(wkc[:], wk_v[:, :, pr * P:(pr + 1) * P])
                    psk = psMM.tile([P, S_OWN], F32, tag="mm")
                    for c in range(CH):
                        nc.tensor.matmul(
                            psk[:], wkc[:, c, :], xnt1[:, c, :],
                            start=(c == 0), stop=(c == CH - 1),
                        )
                    ktev = kvpool.tile([P, S_OWN], FP8, tag="ktev")
                    nc.vector.tensor_copy(ktev[:], psk[:])
                    nc.sync.dma_start(ktv[prh], ktev[:])

                with nc.named_scope(f"L{L}_kv"):
                    for pr in range(PAIRS // 2):
                        k_pair(pr)
                    nc.gpsimd.collective_compute(
                        "AllGather", ALU.bypass, replica_groups=RG,
                        ins=[k_own[L][0][:]], outs=[k_full[L][0][:]],
                    )
                    # V: head-half major; stationary xnt tile reused per hf
                    for hf in range(2):
                        vv = _view(v_own[L][hf], QT, P, HD)
                        for t in range(QT):
                            psv = psMM.tile([P, HD], F32, tag="mm")
                            for c in range(CH):
                                nc.tensor.matmul(
                                    psv[:], xnt1[:, c, t * P:(t + 1) * P],
                                    wv_sb[:, c, hf * HD:(hf + 1) * HD],
                                    start=(c == 0), stop=(c == CH - 1),
                                )
                            vev = kvpool.tile([P, HD], FP8, tag="ktev")
                            nc.vector.tensor_copy(vev[:], psv[:])
                            nc.sync.dma_start(vv[t], vev[:])
                        nc.gpsimd.collective_compute(
                            "AllGather", ALU.bypass, replica_groups=RG,
                            ins=[v_own[L][hf][:]], outs=[v_full[L][hf][:]],
                        )
                    for pr in range(PAIRS // 2, PAIRS):
                        k_pair(pr)
                    nc.gpsimd.collective_compute(
                        "AllGather", ALU.bypass, replica_groups=RG,
                        ins=[k_own[L][1][:]], outs=[k_full[L][1][:]],
                    )

                # ---- Q^T (pairs), overlaps the gathers ---------------------
                with nc.named_scope(f"L{L}_q"):
                    wo_sb = wpool.tile([P, CH, D], BF16, tag="wvo",
                                       name=f"wo_sb{L}")
                    nc.sync.dma_start(wo_sb[:],
                                      wo.rearrange("(c p) n -> p c n", p=P))
                    qt_sb = bpool.tile([P, PAIRS, S_OWN], FP8, tag="qt_sb")
                    for pr in range(PAIRS):
                        wqc = qkvpool.tile([P, CH, P], BF16, tag="wqkc")
                        nc.sync.dma_start(wqc[:], wq_v[:, :, pr * P:(pr + 1) * P])
                        psq = psMM.tile([P, S_OWN], F32, tag="mm")
                        for c in range(CH):
                            nc.tensor.matmul(
                                psq[:], wqc[:, c, :], xnt1[:, c, :],
                                start=(c == 0), stop=(c == CH - 1),
                            )
                        nc.vector.tensor_copy(qt_sb[:, pr, :], psq[:])

                # ---- attention (raw, normalization deferred) ---------------
                with nc.named_scope(f"L{L}_attn"):
                    o_sb = bpool.tile([P, PAIRS, S_OWN], BF16, tag="o_sb")
                    l2s = []
                    for pr in range(PAIRS):
                        hh, cb = divmod(pr, GROUP)
                        kt = kvpool.tile([P, GROUP, S_OWN], FP8, tag="kt")
                        nc.sync.dma_start(
                            kt[:],
                            _view(k_full[L][hh], GROUP, PAIRS // 2, P, S_OWN)
                            [:, cb].rearrange("b p s -> p b s"),
                        )
                        vts = []
                        for par in range(2):
                            vt = vpool.tile([P, KTILES, DK + 1], FP8, tag="vt")
                            nc.vector.memset(vt[:, :, DK:DK + 1], 1.0)
                            lo = cb * P + par * DK
                            nc.sync.dma_start(
                                vt[:, :, 0:DK],
                                _view(v_full[L][hh], GROUP, QT, P, HD)
                                [:, :, :, lo:lo + DK]
                                .rearrange("b t p d -> p (b t) d"),
                            )
                            vts.append(vt)
                        pso = [psO.tile([DK + 1, S_OWN], F32, tag="oo",
                                        name=f"pso{par}")
                               for par in range(2)]
                        for g in range(KTILES // 2):
                            j0, j1 = 2 * g, 2 * g + 1
                            pss = [psS.tile([P, 2 * S_OWN], F32, tag="ss",
                                            name=f"pss{par}")
                                   for par in range(2)]
                            # adjacent par0/par1 on disjoint 64-row groups
                            for u, j in enumerate((j0, j1)):
                                b, jj = divmod(j, GROUP)
                                for par in range(2):
                                    lo = par * DK
                                    nc.tensor.matmul(
                                        pss[par][:, u * S_OWN:(u + 1) * S_OWN],
                                        kt[lo:lo + DK, b, jj * P:(jj + 1) * P],
                                        qt_sb[lo:lo + DK, pr, :],
                                        start=True, stop=True,
                                    )
                            pts = []
                            for par in range(2):
                                pt = ptpool.tile([P, 2 * S_OWN], FP8, tag="pt")
                                nc.scalar.activation(pt[:], pss[par][:],
                                                     AF.Exp, scale=0.125)
                                pts.append(pt)
                            for u, j in enumerate((j0, j1)):
                                for par in range(2):
                                    nc.tensor.matmul(
                                        pso[par][:],
                                        vts[par][:, j, :],
                                        pts[par][:, u * S_OWN:(u + 1) * S_OWN],
                                        start=(g == 0 and u == 0),
                                        stop=(g == KTILES // 2 - 1 and u == 1),
                                    )
                        l2p = apool.tile([DK + 1, S_OWN], F32, tag="l2p",
                                         name=f"l2p{pr}", bufs=PAIRS)
                        nc.vector.memset(l2p[:], 1.0)
                        for par in range(2):
                            nc.vector.tensor_copy(
                                l2p[par * DK:par * DK + 1, :],
                                pso[par][DK:DK + 1, :])
                            nc.vector.tensor_copy(
                                o_sb[par * DK:(par + 1) * DK, pr, :],
                                pso[par][0:DK, :])
                        l2s.append(l2p)

                # ---- normalize + output projection + residual --------------
                with nc.named_scope(f"L{L}_oproj"):
                    for pr in range(PAIRS):
                        lnl = apool.tile([DK + 1, S_OWN], F32, tag="lnl",
                                         bufs=1)
                        nc.scalar.activation(lnl[:], l2s[pr][:], AF.Ln)
                        linv = apool.tile([DK + 1, S_OWN], F32, tag="linv",
                                          bufs=1)
                        nc.scalar.activation(linv[:], lnl[:], AF.Exp, scale=-1.0)
                        psl = psMM.tile([P, S_OWN], F32, tag="mm")
                        nc.tensor.matmul(psl[:], e2_sb[:], linv[:],
                                         start=True, stop=True)
                        nc.vector.tensor_mul(o_sb[:, pr, :], o_sb[:, pr, :],
                                             psl[:])
                    for qt in range(QT):
                        psa = [psMM.tile([P, HD], F32, tag="mm", name=f"psa{hf}")
                               for hf in range(2)]
                        for pr in range(PAIRS):
                            for hf in range(2):
                                nc.tensor.matmul(
                                    psa[hf][:], o_sb[:, pr, qt * P:(qt + 1) * P],
                                    wo_sb[:, pr, hf * HD:(hf + 1) * HD],
                                    start=(pr == 0), stop=(pr == PAIRS - 1),
                                )
                        for hf in range(2):
                            hsl = h_sb[:, qt, hf * HD:(hf + 1) * HD]
                            nc.vector.tensor_add(hsl, hsl, psa[hf][:])

                # ---- FFN sublayer ------------------------------------------
                with nc.named_scope(f"L{L}_ln2"):
                    xnt2 = bpool.tile([P, CH, S_OWN], BF16, tag="xnt")
                    layernorm_transpose(xnt2)

                with nc.named_scope(f"L{L}_ffn1"):
                    ht_sb = bpool.tile([P, FFCH, S_OWN], BF16, tag="ht_sb")
                    for f in range(FFCH):
                        w1c = ffnpool.tile([P, CH, P], BF16, tag="w1c")
                        nc.sync.dma_start(
                            w1c[:], w1h[f].rearrange("(c p) f -> p c f", p=P)
                        )
                        psh = psMM.tile([P, S_OWN], F32, tag="mm")
                        for c in range(CH):
                            nc.tensor.matmul(
                                psh[:], w1c[:, c, :], xnt2[:, c, :],
                                start=(c == 0), stop=(c == CH - 1),
                            )
                        nc.vector.tensor_scalar_max(ht_sb[:, f, :], psh[:], 0.0)

                with nc.named_scope(f"L{L}_ffn2"):
                    for qt in range(QT):
                        psf = [psMM.tile([P, HD], F32, tag="mm", name=f"psf{hf}")
                               for hf in range(2)]
                        for f in range(FFCH):
                            for hf in range(2):
                                nc.tensor.matmul(
                                    psf[hf][:],
                                    ht_sb[:, f, qt * P:(qt + 1) * P],
                                    w2r[:, f, hf * HD:(hf + 1) * HD],
                                    start=(f == 0), stop=(f == FFCH - 1),
                                )
                        for hf in range(2):
                            hsl = h_sb[:, qt, hf * HD:(hf + 1) * HD]
                            nc.vector.tensor_add(hsl, hsl, psf[hf][:])

            # ---- final LN -> output ----------------------------------
            with nc.named_scope("lnf"):
                out_v = out.rearrange("(t p) d -> p t d", p=P)
                for qt in range(QT):
                    hsl = h_sb[:, qt, :]
                    negmu, rstd = layernorm_tile(hsl, "lnf")
                    ot = lpool.tile([P, D], F32, tag="lnf_out")
                    nc.vector.tensor_scalar(
                        ot[:], hsl, negmu[:], rstd[:], ALU.add, ALU.mult
                    )
                    nc.sync.dma_start(out_v[:, qt, :], ot[:])

    _split_multiwaits(nc)
    return nc


_CACHED = {}


def _get_program():
    if "nc" not in _CACHED:
        _CACHED["nc"] = build_program()
    return _CACHED["nc"]


def make_in_maps(inputs):
    x = np.asarray(inputs["x"], np.float32)
    bf = ml_dtypes.bfloat16
    w1 = np.asarray(inputs["w1"], np.float32)
    w1hm = np.ascontiguousarray(
        w1.astype(bf).reshape(D, FFCH, P).transpose(1, 0, 2)
    )
    e2m = np.zeros((DK + 1, P), np.float32)
    e2m[0, 0:DK] = 1.0
    e2m[DK, DK:P] = 1.0
    common = {
        "wq": np.asarray(inputs["wq"], np.float32).astype(bf),
        "wk": np.asarray(inputs["wk"], np.float32).astype(bf),
        "wv": np.asarray(inputs["wv"], np.float32).astype(bf),
        "wo": np.asarray(inputs["wo"], np.float32).astype(bf),
        "w1h": w1hm,
        "w2": np.asarray(inputs["w2"], np.float32).astype(bf),
        "e2": e2m,
        "ident": np.eye(P, dtype=bf),
    }
    xr = x.reshape(B * S, D)
    in_maps = []
    for c in range(NCORES):
        m = dict(common)
        m["x_own"] = np.ascontiguousarray(xr[c * S_OWN:(c + 1) * S_OWN])
        in_maps.append(m)
    return in_maps


def kernel(**inputs):
    in_maps = make_in_maps(inputs)
    nc = _get_program()
    res = run_bass_kernel_spmd(nc, in_maps, list(range(NCORES)))
    full = np.concatenate([res.results[c]["out"] for c in range(NCORES)], axis=0)
    return full.reshape(B, S, D).astype(np.float32)
